# revision 1
# baseline (speedup 1.0000x reference)
"""Trainium2 Bass kernel for LlamaRALAAttention (B=2, S=4096, HID=2048, NH=16, NKV=4, HD=128).

Sharding: 8 cores = DP(batch=2) x TP(kv-head groups=4). Core c handles batch c//4,
kv group c%4 (4 q heads + 1 kv head). Softmax/mean over S stay core-local.
o_proj partials are summed on host (the only cross-core reduction).

Pipeline (per core, "everything transposed" layout):
  xT [HID,S] host-pretransposed, bf16. Projections stream xT chunks as moving operand.
  q path in [d,s] layout: q^T = Wq_h^T @ xT, RoPE via R-matmul + cos/sin mults,
    kappa=exp(min(x,0))+max(x,0) -> QkT (bf16, resident).
  k/v path in [s,d] layout: lhsT=xT tile (stationary), rhs=[Wk|Wv]; RoPE on free dim;
    kappa -> Kk_sd, v_sd (bf16, resident). KkT via PE transpose.
  Qg = mean_s Qk (DVE free-dim reduce); logits via per-s-tile matvecs (lhsT=KkT tile);
  softmax with exact global max (PE transpose + ones-matmul broadcasts, all on-chip);
  outer = (alpha*Kk)^T @ v (PE accumulate); result^T = outer^T.T... lhsT=outer, rhs=QkT;
  ctx^T = phiT * result^T; o_proj: lhsT=ctx^T tiles, rhs=Wo rows -> partial out [S, 2048].
"""

import sys

sys.path.insert(0, "/opt/trn_rl_repo")

import numpy as np
import ml_dtypes

import concourse.bass as bass
import concourse.mybir as mybir
import concourse.tile as tile
from concourse import bacc
from concourse.bass_utils import run_bass_kernel_spmd
from concourse.masks import make_identity

P = 128
S = 4096
HID = 2048
HD = 128
NHL = 4            # q heads per core
KO = HID // P      # 16 contraction subtiles
CS = 512           # token chunk size
NCH = S // CS      # 8 chunks
NST = S // P       # 32 s-tiles
ROPE_THETA = 10000.0

F32 = mybir.dt.float32
BF16 = mybir.dt.bfloat16
BF = ml_dtypes.bfloat16

_CACHE = {}


def _build():
    nc = bacc.Bacc("TRN2", target_bir_lowering=False, debug=False, num_devices=8)

    xT = nc.dram_tensor("xT", [HID, S], BF16, kind="ExternalInput").ap()
    cosT = nc.dram_tensor("cosT", [P, S], F32, kind="ExternalInput").ap()
    sinT = nc.dram_tensor("sinT", [P, S], F32, kind="ExternalInput").ap()
    cos_sd = nc.dram_tensor("cos_sd", [S, HD], F32, kind="ExternalInput").ap()
    sin_sd = nc.dram_tensor("sin_sd", [S, HD], F32, kind="ExternalInput").ap()
    Wq = nc.dram_tensor("Wq", [HID, NHL * HD], BF16, kind="ExternalInput").ap()
    Wkv = nc.dram_tensor("Wkv", [HID, 2 * HD], BF16, kind="ExternalInput").ap()
    Wphi = nc.dram_tensor("Wphi", [HID, NHL * HD], BF16, kind="ExternalInput").ap()
    Wo = nc.dram_tensor("Wo", [NHL * HD, HID], BF16, kind="ExternalInput").ap()
    bphi = nc.dram_tensor("bphi", [NHL * HD], F32, kind="ExternalInput").ap()
    RT = nc.dram_tensor("RT", [P, P], BF16, kind="ExternalInput").ap()
    out = nc.dram_tensor("out", [S, HID], F32, kind="ExternalOutput").ap()

    xT_r = xT.rearrange("(ko p) s -> p ko s", p=P)
    Wq_r = Wq.rearrange("(ko p) m -> p ko m", p=P)
    Wkv_r = Wkv.rearrange("(ko p) m -> p ko m", p=P)
    Wphi_r = Wphi.rearrange("(ko p) m -> p ko m", p=P)
    Wo_r = Wo.rearrange("(h p) n -> p h n", p=P)
    cos_sd_r = cos_sd.rearrange("(t p) d -> p t d", p=P)
    sin_sd_r = sin_sd.rearrange("(t p) d -> p t d", p=P)
    bphi_r = bphi.rearrange("(h p) -> p h", p=P)
    out_r = out.rearrange("(t p) n -> p t n", p=P)

    from contextlib import ExitStack
    with tile.TileContext(nc) as tc, ExitStack() as es:
        # ---- pools ----
        res = es.enter_context(tc.tile_pool(name="res", bufs=1))        # residents
        wts = es.enter_context(tc.tile_pool(name="wts", bufs=2))        # big weights, shared slots
        xp = es.enter_context(tc.tile_pool(name="xp", bufs=3))          # xT chunks
        stream = es.enter_context(tc.tile_pool(name="stream", bufs=2))  # big per-chunk tiles
        stream3 = es.enter_context(tc.tile_pool(name="stream3", bufs=3))  # small per-chunk tiles
        small = es.enter_context(tc.tile_pool(name="small", bufs=4))    # tiny tiles
        pq = es.enter_context(tc.tile_pool(name="pq", bufs=3, space="PSUM"))    # [128,512] proj
        pr = es.enter_context(tc.tile_pool(name="pr", bufs=1, space="PSUM"))    # [128,512] rot/result
        po = es.enter_context(tc.tile_pool(name="po", bufs=2, space="PSUM"))    # [128,512] out
        pmix = es.enter_context(tc.tile_pool(name="pmix", bufs=2, space="PSUM"))  # shared small

        # ---- residents / weights ----
        Wkv_sb = res.tile([P, KO, 2 * HD], BF16)
        nc.sync.dma_start(Wkv_sb[:], Wkv_r)
        Wq_sb = wts.tile([P, KO, NHL * HD], BF16, tag="big")
        RT_sb = res.tile([P, P], BF16)
        nc.sync.dma_start(RT_sb[:], RT)
        bphi_sb = res.tile([P, NHL], F32)
        nc.sync.dma_start(bphi_sb[:], bphi_r)

        ident_bf = res.tile([P, P], BF16)
        make_identity(nc, ident_bf[:])
        ident_f32 = res.tile([P, P], F32)
        make_identity(nc, ident_f32[:])
        ones_f32 = res.tile([P, 1], F32)
        nc.vector.memset(ones_f32[:], 1.0)
        onesr_f32 = res.tile([1, P], F32)
        nc.vector.memset(onesr_f32[:], 1.0)
        negr_f32 = res.tile([1, P], F32)
        nc.vector.memset(negr_f32[:], -1.0)

        QkT = res.tile([P, NHL, S], BF16)       # 32KB/part
        KkT = res.tile([P, S], BF16)            # 8KB/part
        Kk_sd = res.tile([P, NST, HD], BF16)    # 8KB/part
        v_sd = res.tile([P, NST, HD], BF16)     # 8KB/part
        qg_parts = res.tile([P, NHL, NCH], F32)
        outer_bf = res.tile([P, NHL, HD], BF16)
        alpha_sd = res.tile([P, NHL, NST], F32)
        logits_sd = res.tile([P, NHL, NST], F32)

        # ================= phase A: q/k/v projections + rope + kappa =================
        for c in range(NCH):
            xt = xp.tile([P, KO, CS], BF16, tag="xt")
            nc.sync.dma_start(xt[:], xT_r[:, :, c * CS:(c + 1) * CS])
            cs_t = stream.tile([P, CS], F32, tag="cosT")
            nc.sync.dma_start(cs_t[:], cosT[:, c * CS:(c + 1) * CS])
            sn_t = stream.tile([P, CS], F32, tag="sinT")
            nc.sync.dma_start(sn_t[:], sinT[:, c * CS:(c + 1) * CS])
            csd = stream.tile([P, 4, HD], F32, tag="cossd")
            nc.sync.dma_start(csd[:], cos_sd_r[:, c * 4:(c + 1) * 4, :])
            ssd = stream.tile([P, 4, HD], F32, tag="sinsd")
            nc.sync.dma_start(ssd[:], sin_sd_r[:, c * 4:(c + 1) * 4, :])

            # ---- k + v for the 4 s-tiles of this chunk ----
            for st in range(4):
                stg = c * 4 + st
                pskv = pmix.tile([P, 2 * HD], F32, tag="mix")
                for ko in range(KO):
                    nc.tensor.matmul(
                        pskv[:], xt[:, ko, st * P:(st + 1) * P], Wkv_sb[:, ko, :],
                        start=(ko == 0), stop=(ko == KO - 1))
                k_ps = pskv[:, :HD]
                nc.vector.tensor_copy(v_sd[:, stg, :], pskv[:, HD:])
                # rope-k in [s,d]: rot on free halves
                kr = stream3.tile([P, HD], F32, tag="kr")
                nc.vector.tensor_mul(kr[:], k_ps, csd[:, st, :])
                ta = stream3.tile([P, 64], F32, tag="ta")
                nc.vector.tensor_mul(ta[:], k_ps[:, 64:], ssd[:, st, :64])
                nc.vector.tensor_sub(kr[:, :64], kr[:, :64], ta[:])
                tb = stream3.tile([P, 64], F32, tag="tb")
                nc.vector.tensor_mul(tb[:], k_ps[:, :64], ssd[:, st, 64:])
                nc.vector.tensor_add(kr[:, 64:], kr[:, 64:], tb[:])
                # kappa
                mk = stream3.tile([P, HD], F32, tag="mk")
                nc.gpsimd.tensor_scalar_min(mk[:], kr[:], 0.0)
                ek = stream3.tile([P, HD], F32, tag="ek")
                nc.scalar.activation(ek[:], mk[:], mybir.ActivationFunctionType.Exp)
                nc.vector.scalar_tensor_tensor(
                    Kk_sd[:, stg, :], kr[:], 0.0, ek[:],
                    mybir.AluOpType.max, mybir.AluOpType.add)
                # KkT via PE transpose (bf16 in -> fp32 psum -> bf16 sbuf)
                pst = pmix.tile([P, P], BF16, tag="mix")
                nc.tensor.transpose(pst[:], Kk_sd[:, stg, :], ident_bf[:])
                nc.vector.tensor_copy(KkT[:, stg * P:(stg + 1) * P], pst[:])

            if c == 0:
                nc.sync.dma_start(Wq_sb[:], Wq_r)
            # ---- q heads ----
            for h in range(NHL):
                psq = pq.tile([P, CS], F32, tag="psq")
                for ko in range(KO):
                    nc.tensor.matmul(
                        psq[:], Wq_sb[:, ko, h * HD:(h + 1) * HD], xt[:, ko, :],
                        start=(ko == 0), stop=(ko == KO - 1))
                # sin is 64-periodic over d, so rot(q)*sin == rot(q*sin):
                # multiply by sin BEFORE the rotation matmul (saves the psum copy)
                qs = stream3.tile([P, CS], BF16, tag="qbf")
                nc.vector.tensor_mul(qs[:], psq[:], sn_t[:])
                psr = pr.tile([P, CS], F32, tag="psr")
                nc.tensor.matmul(psr[:], RT_sb[:], qs[:], start=True, stop=True)
                qro = stream.tile([P, CS], F32, tag="qro")
                nc.vector.tensor_mul(qro[:], psq[:], cs_t[:])
                nc.vector.tensor_add(qro[:], qro[:], psr[:])
                # kappa -> QkT
                mq = stream.tile([P, CS], F32, tag="mq")
                nc.gpsimd.tensor_scalar_min(mq[:], qro[:], 0.0)
                eq = stream.tile([P, CS], F32, tag="eq")
                nc.scalar.activation(eq[:], mq[:], mybir.ActivationFunctionType.Exp)
                nc.vector.scalar_tensor_tensor(
                    QkT[:, h, c * CS:(c + 1) * CS], qro[:], 0.0, eq[:],
                    mybir.AluOpType.max, mybir.AluOpType.add)
                # Qg partial
                nc.vector.tensor_reduce(
                    qg_parts[:, h, c:c + 1], QkT[:, h, c * CS:(c + 1) * CS],
                    mybir.AxisListType.X, mybir.AluOpType.add)

        Wphi_sb = wts.tile([P, KO, NHL * HD], BF16, tag="big")
        nc.sync.dma_start(Wphi_sb[:], Wphi_r)
        Wo_sb = wts.tile([P, NHL, HID], BF16, tag="big")
        nc.sync.dma_start(Wo_sb[:], Wo_r)

        # ================= phase B: Qg, logits, softmax, outer =================
        qg_bf = small.tile([P, NHL], BF16, tag="qgbf")
        qg_f = small.tile([P, NHL], F32, tag="qgf")
        for h in range(NHL):
            nc.vector.tensor_reduce(
                qg_f[:, h:h + 1], qg_parts[:, h, :],
                mybir.AxisListType.X, mybir.AluOpType.add)
        nc.vector.tensor_scalar_mul(qg_bf[:], qg_f[:], 1.0 / S)

        # logits[s] per head: lhsT = KkT tile [d, s-tile], rhs = qg column
        for st in range(NST):
            psl = pmix.tile([P, NHL], F32, tag="mix")
            for h in range(NHL):
                nc.tensor.matmul(
                    psl[:, h:h + 1], KkT[:, st * P:(st + 1) * P],
                    qg_bf[:, h:h + 1], start=True, stop=True)
            nc.vector.tensor_copy(
                logits_sd.rearrange("p h t -> p t h")[:, st, :], psl[:])

        for h in range(NHL):
            lg = logits_sd[:, h, :]                       # [128, 32]
            pmax = small.tile([P, 1], F32, tag="pmax")
            nc.vector.tensor_reduce(pmax[:], lg, mybir.AxisListType.X, mybir.AluOpType.max)
            # global max: transpose pmax -> [1,128], reduce, negate-broadcast back
            pmt = pmix.tile([1, P], F32, tag="mix")
            nc.tensor.transpose(pmt[:], pmax[:], ident_f32[:])
            gmax = small.tile([1, 1], F32, tag="gmax")
            nc.vector.tensor_reduce(gmax[:], pmt[:], mybir.AxisListType.X, mybir.AluOpType.max)
            pngm = pmix.tile([P, 1], F32, tag="mix")
            nc.tensor.matmul(pngm[:], negr_f32[:], gmax[:], start=True, stop=True)
            ngm = small.tile([P, 1], F32, tag="ngm")
            nc.vector.tensor_copy(ngm[:], pngm[:])
            # e = exp(l - gmax), per-partition sums via accum_out
            e_sd = small.tile([P, NST], F32, tag="esd")
            srow = small.tile([P, 1], F32, tag="srow")
            nc.scalar.activation(e_sd[:], lg, mybir.ActivationFunctionType.Exp,
                                 bias=ngm[:], accum_out=srow[:])
            # total = sum_p srow  (fp32 matmul), then rcp broadcast
            ptot = pmix.tile([1, 1], F32, tag="mix")
            nc.tensor.matmul(ptot[:], srow[:], ones_f32[:], start=True, stop=True)
            rcp = small.tile([1, 1], F32, tag="rcp")
            nc.vector.reciprocal(rcp[:], ptot[:])
            prc = pmix.tile([P, 1], F32, tag="mix")
            nc.tensor.matmul(prc[:], onesr_f32[:], rcp[:], start=True, stop=True)
            rcpb = small.tile([P, 1], F32, tag="rcpb")
            nc.vector.tensor_copy(rcpb[:], prc[:])
            nc.vector.tensor_scalar(
                alpha_sd[:, h, :], e_sd[:], rcpb[:], float(S),
                mybir.AluOpType.mult, mybir.AluOpType.mult)

        # outer[h] = sum_st (alpha*Kk_tile)^T... lhsT=KkA [s,d], rhs=v [s,f]
        for h in range(NHL):
            pso = pmix.tile([P, HD], F32, tag="mix")
            for st in range(NST):
                kka = stream3.tile([P, HD], BF16, tag="kka")
                nc.vector.tensor_scalar_mul(
                    kka[:], Kk_sd[:, st, :], alpha_sd[:, h, st:st + 1])
                nc.tensor.matmul(pso[:], kka[:], v_sd[:, st, :],
                                 start=(st == 0), stop=(st == NST - 1))
            nc.vector.tensor_copy(outer_bf[:, h, :], pso[:])

        # ================= phase C: result_attn, ctx, o_proj =================
        for c in range(NCH):
            xt = xp.tile([P, KO, CS], BF16, tag="xt")
            nc.sync.dma_start(xt[:], xT_r[:, :, c * CS:(c + 1) * CS])
            ctx_bf = stream.tile([P, NHL, CS], BF16, tag="ctx")
            for h in range(NHL):
                psp = pq.tile([P, CS], F32, tag="psq")
                for ko in range(KO):
                    nc.tensor.matmul(
                        psp[:], Wphi_sb[:, ko, h * HD:(h + 1) * HD], xt[:, ko, :],
                        start=(ko == 0), stop=(ko == KO - 1))
                phiT = stream.tile([P, CS], F32, tag="phiT")
                nc.scalar.activation(phiT[:], psp[:], mybir.ActivationFunctionType.Identity, bias=bphi_sb[:, h:h + 1])
                psr = pr.tile([P, CS], F32, tag="psr")
                nc.tensor.matmul(psr[:], outer_bf[:, h, :],
                                 QkT[:, h, c * CS:(c + 1) * CS], start=True, stop=True)
                for st in range(4):
                    nc.vector.tensor_mul(
                        ctx_bf[:, h, st * P:(st + 1) * P],
                        phiT[:, st * P:(st + 1) * P], psr[:, st * P:(st + 1) * P])
            # o_proj for the 4 s-tiles of this chunk
            for st in range(4):
                stg = c * 4 + st
                for n in range(4):
                    pso2 = po.tile([P, 512], F32, tag="psout")
                    for h in range(NHL):
                        nc.tensor.matmul(
                            pso2[:], ctx_bf[:, h, st * P:(st + 1) * P],
                            Wo_sb[:, h, n * 512:(n + 1) * 512],
                            start=(h == 0), stop=(h == NHL - 1))
                    ob = stream.tile([P, 512], F32, tag="ob")
                    if (st + n) % 2 == 0:
                        nc.vector.tensor_copy(ob[:], pso2[:])
                    else:
                        nc.scalar.copy(ob[:], pso2[:])
                    nc.sync.dma_start(out_r[:, stg, n * 512:(n + 1) * 512], ob[:])

    nc.compile()
    return nc


def _host_prep(hidden_states, position_ids, Wq, Wk, Wv, Wo, Wphi, bphi):
    B = hidden_states.shape[0]
    # rope tables (match reference fp32 math)
    inv_freq = (1.0 / (ROPE_THETA ** (np.arange(0, HD, 2, dtype=np.float32) / HD))).astype(np.float32)
    in_maps = []
    Rm = np.zeros((P, P), dtype=np.float32)
    Rm[np.arange(64), np.arange(64) + 64] = -1.0
    Rm[np.arange(64) + 64, np.arange(64)] = 1.0
    RT_np = np.ascontiguousarray(Rm.T).astype(BF)
    for b in range(B):
        freqs = position_ids[b].astype(np.float32)[:, None] * inv_freq[None, :]
        emb = np.concatenate([freqs, freqs], axis=1)          # [S, 128]
        cos_b = np.cos(emb).astype(np.float32)
        sin_b = np.sin(emb).astype(np.float32)
        xT_b = np.ascontiguousarray(hidden_states[b].T).astype(BF)
        cosT_b = np.ascontiguousarray(cos_b.T)
        sinT_b = np.ascontiguousarray(sin_b.T)
        for g in range(4):
            sl4 = slice(g * 512, (g + 1) * 512)
            sl1 = slice(g * 128, (g + 1) * 128)
            in_maps.append({
                "xT": xT_b,
                "cosT": cosT_b, "sinT": sinT_b,
                "cos_sd": cos_b, "sin_sd": sin_b,
                "Wq": np.ascontiguousarray(Wq[:, sl4]).astype(BF),
                "Wkv": np.ascontiguousarray(
                    np.concatenate([Wk[:, sl1], Wv[:, sl1]], axis=1)).astype(BF),
                "Wphi": np.ascontiguousarray(Wphi[:, sl4]).astype(BF),
                "Wo": np.ascontiguousarray(Wo[sl4, :]).astype(BF),
                "bphi": np.ascontiguousarray(bphi[sl4]).astype(np.float32),
                "RT": RT_np,
            })
    return in_maps


def kernel(hidden_states, position_ids, Wq, Wk, Wv, Wo, Wphi, bphi, _trace=False):
    if "nc" not in _CACHE:
        _CACHE["nc"] = _build()
    nc = _CACHE["nc"]
    in_maps = _host_prep(np.asarray(hidden_states), np.asarray(position_ids),
                         np.asarray(Wq), np.asarray(Wk), np.asarray(Wv),
                         np.asarray(Wo), np.asarray(Wphi), np.asarray(bphi))
    res = run_bass_kernel_spmd(nc, in_maps, list(range(8)), trace=_trace)
    _CACHE["last_res"] = res
    B = hidden_states.shape[0]
    out = np.empty((B, S, HID), dtype=np.float32)
    for b in range(B):
        acc = res.results[b * 4 + 0]["out"].astype(np.float32)
        for g in range(1, 4):
            acc = acc + res.results[b * 4 + g]["out"]
        out[b] = acc
    return out



# revision 10
# speedup vs baseline: 1.1139x; 1.1139x over previous
"""Trainium2 Bass kernel for LlamaRALAAttention (B=2, S=4096, HID=2048, NH=16, NKV=4, HD=128).

Sharding: 8 cores = DP(batch=2) x TP(kv-head groups=4). Core c handles batch c//4,
kv group c%4 (4 q heads + 1 kv head). Softmax/mean over S stay core-local.
o_proj partials are written bf16 and summed on host (the only cross-core reduction).

Pipeline (per core, "everything transposed" layout):
  xT [HID,S] host-pretransposed, bf16. Projections stream xT chunks as moving operand.
  q path in [d,s] layout: q^T = Wq_h^T @ xT, RoPE via R-matmul + cos/sin mults,
    kappa=exp(min(x,0))+max(x,0) -> QkT (bf16, resident); Qg partial folded into the
    kappa op via accum_out.
  k/v path in [s,d] layout: lhsT=xT tile (stationary), rhs=[Wk|Wv]; RoPE on free dim;
    kappa -> Kk_sd, v_sd (bf16, resident). KkT via PE transpose.
  All DVE-fed PE ops (KkT transposes, q-rope matmuls) go through a pending FIFO and are
  issued one matmul-group late, so the in-order PE queue never waits on a DVE chain.
  Logits: per-s-tile matvecs batched over the 4 heads into one PSUM bank; softmax
  (exact global max) batched over heads and interleaved into the first phi chunk;
  outer = (alpha*Kk)^T @ v with alpha applied via broadcast muls; result^T: lhsT=outer,
  rhs=QkT; ctx^T = phiT * result^T; o_proj one chunk behind phi/result so the ctx DVE
  muls never stall PE.
"""

import sys

sys.path.insert(0, "/opt/trn_rl_repo")

import numpy as np
import ml_dtypes

import concourse.bass as bass
import concourse.mybir as mybir
import concourse.tile as tile
from concourse import bacc
from concourse.bass_utils import run_bass_kernel_spmd
from concourse.masks import make_identity

P = 128
S = 4096
HID = 2048
HD = 128
NHL = 4            # q heads per core
KO = HID // P      # 16 contraction subtiles
CS = 512           # token chunk size
NCH = S // CS      # 8 chunks
NST = S // P       # 32 s-tiles
HSTEP = NST // 2
ROPE_THETA = 10000.0

F32 = mybir.dt.float32
BF16 = mybir.dt.bfloat16
BF = ml_dtypes.bfloat16

_CACHE = {}


def _build():
    nc = bacc.Bacc("TRN2", target_bir_lowering=False, debug=False, num_devices=8)

    xT = nc.dram_tensor("xT", [HID, S], BF16, kind="ExternalInput").ap()
    cosT = nc.dram_tensor("cosT", [P, S], F32, kind="ExternalInput").ap()
    sinT = nc.dram_tensor("sinT", [P, S], F32, kind="ExternalInput").ap()
    cos_sd = nc.dram_tensor("cos_sd", [S, HD], F32, kind="ExternalInput").ap()
    sin_sd = nc.dram_tensor("sin_sd", [S, HD], F32, kind="ExternalInput").ap()
    Wq = nc.dram_tensor("Wq", [HID, NHL * HD], BF16, kind="ExternalInput").ap()
    Wkv = nc.dram_tensor("Wkv", [HID, 2 * HD], BF16, kind="ExternalInput").ap()
    Wphi = nc.dram_tensor("Wphi", [HID, NHL * HD], BF16, kind="ExternalInput").ap()
    Wo = nc.dram_tensor("Wo", [NHL * HD, HID], BF16, kind="ExternalInput").ap()
    bphi = nc.dram_tensor("bphi", [NHL * HD], F32, kind="ExternalInput").ap()
    RT = nc.dram_tensor("RT", [P, P], BF16, kind="ExternalInput").ap()
    out = nc.dram_tensor("out", [S, HID], BF16, kind="ExternalOutput").ap()

    xT_r = xT.rearrange("(ko p) s -> p ko s", p=P)
    Wq_r = Wq.rearrange("(ko p) m -> p ko m", p=P)
    Wkv_r = Wkv.rearrange("(ko p) m -> p ko m", p=P)
    Wphi_r = Wphi.rearrange("(ko p) m -> p ko m", p=P)
    Wo_r = Wo.rearrange("(h p) n -> p h n", p=P)
    cos_sd_r = cos_sd.rearrange("(t p) d -> p t d", p=P)
    sin_sd_r = sin_sd.rearrange("(t p) d -> p t d", p=P)
    bphi_r = bphi.rearrange("(h p) -> p h", p=P)
    out_r = out.rearrange("(t p) n -> p t n", p=P)

    from contextlib import ExitStack
    with tile.TileContext(nc) as tc, ExitStack() as es:
        # ---- pools ----
        res = es.enter_context(tc.tile_pool(name="res", bufs=1))        # residents
        wts = es.enter_context(tc.tile_pool(name="wts", bufs=2))        # big weights, shared slots
        xp = es.enter_context(tc.tile_pool(name="xp", bufs=2))          # xT chunks
        stream = es.enter_context(tc.tile_pool(name="stream", bufs=2))  # big per-chunk tiles
        stream3 = es.enter_context(tc.tile_pool(name="stream3", bufs=3))  # small per-chunk tiles
        small = es.enter_context(tc.tile_pool(name="small", bufs=2))    # tiny tiles
        # PSUM: 8 banks total. pa: all projections (kv/q/phi); pr: rope+result;
        # po: o_proj out; pmix: transposes/logits/softmax/outer (strictly sequential).
        pa = es.enter_context(tc.tile_pool(name="pa", bufs=3, space="PSUM"))
        pr = es.enter_context(tc.tile_pool(name="pr", bufs=2, space="PSUM"))
        po = es.enter_context(tc.tile_pool(name="po", bufs=2, space="PSUM"))
        pmix = es.enter_context(tc.tile_pool(name="pmix", bufs=1, space="PSUM"))

        # ---- residents / weights ----
        Wkv_sb = res.tile([P, KO, 2 * HD], BF16)
        nc.sync.dma_start(Wkv_sb[:], Wkv_r)
        RT_sb = res.tile([P, P], BF16)
        nc.sync.dma_start(RT_sb[:], RT)
        bphi_sb = res.tile([P, NHL], F32)
        nc.sync.dma_start(bphi_sb[:], bphi_r)
        Wq_sb = wts.tile([P, KO, NHL * HD], BF16, tag="big")

        ident_bf = res.tile([P, P], BF16)
        make_identity(nc, ident_bf[:])
        ident_f32 = res.tile([P, P], F32)
        make_identity(nc, ident_f32[:])
        ones_f32 = res.tile([P, 1], F32)
        nc.vector.memset(ones_f32[:], 1.0)
        onesr_f32 = res.tile([1, P], F32)
        nc.vector.memset(onesr_f32[:], 1.0)
        negr_f32 = res.tile([1, P], F32)
        nc.vector.memset(negr_f32[:], -1.0)

        QkT = res.tile([P, NHL, S], BF16)       # 32KB/part
        KkT = res.tile([P, S], BF16)            # 8KB/part
        Kk_sd = res.tile([P, NST, HD], BF16)    # 8KB/part
        v_sd = res.tile([P, NST, HD], BF16)     # 8KB/part
        qg_parts = res.tile([P, NHL, NCH], F32)
        outer_bf = res.tile([P, NHL, HD], BF16)
        alpha_bf = res.tile([P, NHL, NST], BF16)
        logits_sd = res.tile([P, NHL, NST], F32)

        # ================= phase A =================
        # Pending PE work that depends on a DVE chain; each entry is issued one
        # matmul-group later so the in-order PE queue never stalls.
        pend = []

        def flush_pending():
            if pend:
                pend.pop(0)()

        def make_transpose(stg):
            def f():
                pst = pmix.tile([P, P], BF16, tag="mix", name="pst")
                nc.tensor.transpose(pst[:], Kk_sd[:, stg, :], ident_bf[:])
                nc.any.tensor_copy(KkT[:, stg * P:(stg + 1) * P], pst[:])
            return f

        def make_rope(h, c, psq, qs, cs_t):
            def f():
                psr = pr.tile([P, CS], F32, tag="psr", name="psr")
                nc.tensor.matmul(psr[:], RT_sb[:], qs[:], start=True, stop=True)
                qro = stream.tile([P, CS], F32, tag="qro", name="qro")
                nc.vector.tensor_mul(qro[:], psq[:], cs_t[:])
                nc.vector.tensor_add(qro[:], qro[:], psr[:])
                # kappa -> QkT, Qg partial folded in via accum_out
                mq = stream.tile([P, CS], F32, tag="mq", name="mq")
                nc.gpsimd.tensor_scalar_min(mq[:], qro[:], 0.0)
                eq = stream.tile([P, CS], F32, tag="eq", name="eq")
                nc.scalar.activation(eq[:], mq[:], mybir.ActivationFunctionType.Exp)
                nc.vector.scalar_tensor_tensor(
                    QkT[:, h, c * CS:(c + 1) * CS], qro[:], 0.0, eq[:],
                    mybir.AluOpType.max, mybir.AluOpType.add,
                    accum_out=qg_parts[:, h, c:c + 1])
            return f

        xt_c0_phaseC = [None]

        for c in range(NCH):
            xt = xp.tile([P, KO, CS], BF16, tag="xt", name="xt")
            if c == 0:
                # split so the first kv matmul group starts as early as possible
                for st in range(4):
                    nc.sync.dma_start(xt[:, :, st * P:(st + 1) * P],
                                      xT_r[:, :, st * P:(st + 1) * P])
            else:
                nc.sync.dma_start(xt[:, :, :CS // 2], xT_r[:, :, c * CS:c * CS + CS // 2])
                nc.sync.dma_start(xt[:, :, CS // 2:], xT_r[:, :, c * CS + CS // 2:(c + 1) * CS])
            csd = stream.tile([P, 4, HD], F32, tag="cossd", name="csd")
            nc.sync.dma_start(csd[:], cos_sd_r[:, c * 4:(c + 1) * 4, :])
            ssd = stream.tile([P, 4, HD], F32, tag="sinsd", name="ssd")
            nc.sync.dma_start(ssd[:], sin_sd_r[:, c * 4:(c + 1) * 4, :])
            if c == 0:
                nc.sync.dma_start(Wq_sb[:, :KO // 2, :], Wq_r[:, :KO // 2, :])
                nc.sync.dma_start(Wq_sb[:, KO // 2:, :], Wq_r[:, KO // 2:, :])
            cs_t = stream.tile([P, CS], F32, tag="cosT", name="cs_t")
            nc.sync.dma_start(cs_t[:], cosT[:, c * CS:(c + 1) * CS])
            sn_t = stream.tile([P, CS], F32, tag="sinT", name="sn_t")
            nc.sync.dma_start(sn_t[:], sinT[:, c * CS:(c + 1) * CS])
            if c == NCH - 1:
                # prefetch phase-C weights + first phase-C x chunk during the last
                # phase-A chunk's compute
                Wphi_sb = wts.tile([P, KO, NHL * HD], BF16, tag="big")
                nc.sync.dma_start(Wphi_sb[:], Wphi_r)
                xtc0 = xp.tile([P, KO, CS], BF16, tag="xt", name="xt")
                nc.sync.dma_start(xtc0[:, :, :CS // 2], xT_r[:, :, :CS // 2])
                nc.sync.dma_start(xtc0[:, :, CS // 2:], xT_r[:, :, CS // 2:CS])
                xt_c0_phaseC[0] = xtc0

            # ---- k + v for the 4 s-tiles of this chunk ----
            for st in range(4):
                stg = c * 4 + st
                pskv = pa.tile([P, 2 * HD], F32, tag="pa", name="pskv")
                for ko in range(KO):
                    nc.tensor.matmul(
                        pskv[:], xt[:, ko, st * P:(st + 1) * P], Wkv_sb[:, ko, :],
                        start=(ko == 0), stop=(ko == KO - 1))
                flush_pending()
                k_ps = pskv[:, :HD]
                nc.any.tensor_copy(v_sd[:, stg, :], pskv[:, HD:])
                # rope-k in [s,d]: rot on free halves
                kr = stream3.tile([P, HD], F32, tag="kr", name="kr")
                nc.vector.tensor_mul(kr[:], k_ps, csd[:, st, :])
                ta = stream3.tile([P, 64], F32, tag="ta", name="ta")
                nc.vector.tensor_mul(ta[:], k_ps[:, 64:], ssd[:, st, :64])
                nc.vector.tensor_sub(kr[:, :64], kr[:, :64], ta[:])
                tb = stream3.tile([P, 64], F32, tag="tb", name="tb")
                nc.vector.tensor_mul(tb[:], k_ps[:, :64], ssd[:, st, 64:])
                nc.vector.tensor_add(kr[:, 64:], kr[:, 64:], tb[:])
                # kappa
                mk = stream3.tile([P, HD], F32, tag="mk", name="mk")
                nc.gpsimd.tensor_scalar_min(mk[:], kr[:], 0.0)
                ek = stream3.tile([P, HD], F32, tag="ek", name="ek")
                nc.scalar.activation(ek[:], mk[:], mybir.ActivationFunctionType.Exp)
                nc.vector.scalar_tensor_tensor(
                    Kk_sd[:, stg, :], kr[:], 0.0, ek[:],
                    mybir.AluOpType.max, mybir.AluOpType.add)
                pend.append(make_transpose(stg))

            # ---- q heads ----
            for h in range(NHL):
                psq = pa.tile([P, CS], F32, tag="pa", name="psq")
                for ko in range(KO):
                    nc.tensor.matmul(
                        psq[:], Wq_sb[:, ko, h * HD:(h + 1) * HD], xt[:, ko, :],
                        start=(ko == 0), stop=(ko == KO - 1))
                flush_pending()
                # sin is 64-periodic over d, so rot(q)*sin == rot(q*sin):
                # multiply by sin BEFORE the rotation matmul (saves the psum copy)
                qs = stream3.tile([P, CS], BF16, tag="qbf", name="qs")
                nc.vector.tensor_mul(qs[:], psq[:], sn_t[:])
                pend.append(make_rope(h, c, psq, qs, cs_t))

        # ================= phase B + C =================
        Wo_sb = wts.tile([P, NHL, HID], BF16, tag="big")
        nc.sync.dma_start(Wo_sb[:], Wo_r)

        flush_pending()   # rope(h3, c7)

        # tiny sbuf tiles for the batched softmax
        qg_bf = small.tile([P, NHL], BF16, tag="qgbf")
        qg_f = small.tile([P, NHL], F32, tag="qgf")
        pmax4 = small.tile([P, NHL, 1], F32, tag="pmax4")
        gmax4 = small.tile([NHL, 1], F32, tag="gmax4")
        gms = small.tile([1, NHL], F32, tag="gms")
        ngm4 = small.tile([P, NHL], F32, tag="ngm4")
        e4 = small.tile([P, NHL, NST], F32, tag="e4", bufs=1)
        srow4 = small.tile([P, NHL], F32, tag="srow4")
        rcp4 = small.tile([NHL, 1], F32, tag="rcp4")
        rcs = small.tile([1, NHL], F32, tag="rcs")
        rcpb4 = small.tile([P, NHL], F32, tag="rcpb4")

        def issue_logits():
            # Qg finalize (DVE), then logits for all heads, accumulated into one bank
            for h in range(NHL):
                nc.vector.tensor_reduce(
                    qg_f[:, h:h + 1], qg_parts[:, h, :],
                    mybir.AxisListType.X, mybir.AluOpType.add)
            nc.vector.tensor_scalar_mul(qg_bf[:], qg_f[:], 1.0 / S)
            psl = pmix.tile([P, NST, NHL], F32, tag="mix", name="psl")
            for st in range(NST):
                nc.tensor.matmul(
                    psl[:, st, :], KkT[:, st * P:(st + 1) * P], qg_bf[:],
                    start=True, stop=True)
            nc.any.tensor_copy(logits_sd.rearrange("p h t -> p t h")[:], psl[:])
            nc.vector.tensor_reduce(
                pmax4[:], logits_sd[:], mybir.AxisListType.X, mybir.AluOpType.max)

        def issue_softmax1():
            # global max per head: transpose partial maxes, reduce, negate-broadcast
            pmt4 = pmix.tile([NHL, P], F32, tag="mix", name="pmt4")
            nc.tensor.transpose(pmt4[:], pmax4[:, :, 0], ident_f32[:])
            nc.vector.tensor_reduce(
                gmax4[:], pmt4[:], mybir.AxisListType.X, mybir.AluOpType.max)
            gmT = pmix.tile([1, NHL], F32, tag="mix", name="gmT")
            nc.tensor.transpose(gmT[:], gmax4[:], ident_f32[:NHL, :NHL])
            nc.vector.tensor_scalar_mul(gms[:], gmT[:], -1.0)
            pngm4 = pmix.tile([P, NHL], F32, tag="mix", name="pngm4")
            nc.tensor.matmul(pngm4[:], onesr_f32[:], gms[:], start=True, stop=True)
            nc.vector.tensor_copy(ngm4[:], pngm4[:])
            # e = exp(l - gmax) per head, row sums via accum_out
            for h in range(NHL):
                nc.scalar.activation(
                    e4[:, h, :], logits_sd[:, h, :], mybir.ActivationFunctionType.Exp,
                    bias=ngm4[:, h:h + 1], accum_out=srow4[:, h:h + 1])

        def issue_softmax2():
            # totals per head (fp32 matmul), reciprocal, broadcast, alpha = S * e / Z
            ptot4 = pmix.tile([NHL, 1], F32, tag="mix", name="ptot4")
            nc.tensor.matmul(ptot4[:], srow4[:], ones_f32[:], start=True, stop=True)
            nc.vector.reciprocal(rcp4[:], ptot4[:])
            rcT = pmix.tile([1, NHL], F32, tag="mix", name="rcT")
            nc.tensor.transpose(rcT[:], rcp4[:], ident_f32[:NHL, :NHL])
            nc.vector.tensor_copy(rcs[:], rcT[:])
            prc4 = pmix.tile([P, NHL], F32, tag="mix", name="prc4")
            nc.tensor.matmul(prc4[:], onesr_f32[:], rcs[:], start=True, stop=True)
            nc.vector.tensor_copy(rcpb4[:], prc4[:])
            nc.vector.scalar_tensor_tensor(
                alpha_bf[:], e4[:], float(S),
                rcpb4[:].unsqueeze(2).broadcast_to([P, NHL, NST]),
                mybir.AluOpType.mult, mybir.AluOpType.mult)

        def issue_outer(h):
            # alpha (broadcast over d) * Kk in two half-s blocks, accumulate outer
            pso = pmix.tile([P, HD], F32, tag="mix", name="pso")
            for half in range(2):
                st0 = half * HSTEP
                kka = stream3.tile([P, HSTEP, HD], BF16, tag="kka", name="kka", bufs=2)
                ab = alpha_bf[:, h, st0:st0 + HSTEP].unsqueeze(2).broadcast_to([P, HSTEP, HD])
                nc.vector.tensor_mul(kka[:], Kk_sd[:, st0:st0 + HSTEP, :], ab)
                for st in range(HSTEP):
                    nc.tensor.matmul(pso[:], kka[:, st, :], v_sd[:, st0 + st, :],
                                     start=(half == 0 and st == 0),
                                     stop=(half == 1 and st == HSTEP - 1))
            nc.any.tensor_copy(outer_bf[:, h, :], pso[:])

        copy_engines = [
            lambda dst, src: nc.vector.tensor_copy(dst, src),
            lambda dst, src: nc.scalar.copy(dst, src),
        ]
        _ctx_of = {}

        def issue_oproj(c):
            ctx_bf = _ctx_of[c]
            for st in range(4):
                stg = c * 4 + st
                for n in range(4):
                    pso2 = po.tile([P, 512], F32, tag="psout", name="pso2")
                    for h in range(NHL):
                        nc.tensor.matmul(
                            pso2[:], ctx_bf[:, h, st * P:(st + 1) * P],
                            Wo_sb[:, h, n * 512:(n + 1) * 512],
                            start=(h == 0), stop=(h == NHL - 1))
                    ob = stream.tile([P, 512], BF16, tag="ob", name="ob")
                    copy_engines[(st + n) % 2](ob[:], pso2[:])
                    nc.sync.dma_start(out_r[:, stg, n * 512:(n + 1) * 512], ob[:])

        for c in range(NCH):
            if c == 0:
                xt = xt_c0_phaseC[0]
            else:
                xt = xp.tile([P, KO, CS], BF16, tag="xt", name="xt")
                nc.sync.dma_start(xt[:, :, :CS // 2], xT_r[:, :, c * CS:c * CS + CS // 2])
                nc.sync.dma_start(xt[:, :, CS // 2:], xT_r[:, :, c * CS + CS // 2:(c + 1) * CS])
            ctx_bf = stream.tile([P, NHL, CS], BF16, tag="ctx", name="ctx_bf")
            _ctx_of[c] = ctx_bf
            for h in range(NHL):
                psp = pa.tile([P, CS], F32, tag="pa", name="psp")
                for ko in range(KO):
                    nc.tensor.matmul(
                        psp[:], Wphi_sb[:, ko, h * HD:(h + 1) * HD], xt[:, ko, :],
                        start=(ko == 0), stop=(ko == KO - 1))
                if c == 0:
                    # softmax + outer ride inside chunk 0's phi matmul groups
                    if h == 0:
                        issue_logits()
                    elif h == 1:
                        issue_softmax1()
                    elif h == 2:
                        issue_softmax2()
                phiT = stream.tile([P, CS], F32, tag="phiT", name="phiT", bufs=4)
                nc.scalar.activation(phiT[:], psp[:], mybir.ActivationFunctionType.Identity,
                                     bias=bphi_sb[:, h:h + 1])
                if c > 0:
                    psr = pr.tile([P, CS], F32, tag="psr", name="psr")
                    nc.tensor.matmul(psr[:], outer_bf[:, h, :],
                                     QkT[:, h, c * CS:(c + 1) * CS], start=True, stop=True)
                    nc.vector.tensor_mul(ctx_bf[:, h, :], phiT[:], psr[:])
                else:
                    _phiT_c0 = _ctx_of.setdefault("phiT_c0", [])
                    _phiT_c0.append(phiT)
            if c == 0:
                for h in range(NHL):
                    issue_outer(h)
                for h in range(NHL):
                    psr = pr.tile([P, CS], F32, tag="psr", name="psr")
                    nc.tensor.matmul(psr[:], outer_bf[:, h, :],
                                     QkT[:, h, :CS], start=True, stop=True)
                    nc.vector.tensor_mul(ctx_bf[:, h, :], _ctx_of["phiT_c0"][h][:], psr[:])
            else:
                issue_oproj(c - 1)
        issue_oproj(NCH - 1)

    nc.compile()
    return nc


def _host_prep(hidden_states, position_ids, Wq, Wk, Wv, Wo, Wphi, bphi):
    B = hidden_states.shape[0]
    # rope tables (match reference fp32 math)
    inv_freq = (1.0 / (ROPE_THETA ** (np.arange(0, HD, 2, dtype=np.float32) / HD))).astype(np.float32)
    in_maps = []
    Rm = np.zeros((P, P), dtype=np.float32)
    Rm[np.arange(64), np.arange(64) + 64] = -1.0
    Rm[np.arange(64) + 64, np.arange(64)] = 1.0
    RT_np = np.ascontiguousarray(Rm.T).astype(BF)
    for b in range(B):
        freqs = position_ids[b].astype(np.float32)[:, None] * inv_freq[None, :]
        emb = np.concatenate([freqs, freqs], axis=1)          # [S, 128]
        cos_b = np.cos(emb).astype(np.float32)
        sin_b = np.sin(emb).astype(np.float32)
        xT_b = np.ascontiguousarray(hidden_states[b].T).astype(BF)
        cosT_b = np.ascontiguousarray(cos_b.T)
        sinT_b = np.ascontiguousarray(sin_b.T)
        for g in range(4):
            sl4 = slice(g * 512, (g + 1) * 512)
            sl1 = slice(g * 128, (g + 1) * 128)
            in_maps.append({
                "xT": xT_b,
                "cosT": cosT_b, "sinT": sinT_b,
                "cos_sd": cos_b, "sin_sd": sin_b,
                "Wq": np.ascontiguousarray(Wq[:, sl4]).astype(BF),
                "Wkv": np.ascontiguousarray(
                    np.concatenate([Wk[:, sl1], Wv[:, sl1]], axis=1)).astype(BF),
                "Wphi": np.ascontiguousarray(Wphi[:, sl4]).astype(BF),
                "Wo": np.ascontiguousarray(Wo[sl4, :]).astype(BF),
                "bphi": np.ascontiguousarray(bphi[sl4]).astype(np.float32),
                "RT": RT_np,
            })
    return in_maps


def kernel(hidden_states, position_ids, Wq, Wk, Wv, Wo, Wphi, bphi, _trace=False):
    if "nc" not in _CACHE:
        _CACHE["nc"] = _build()
    nc = _CACHE["nc"]
    in_maps = _host_prep(np.asarray(hidden_states), np.asarray(position_ids),
                         np.asarray(Wq), np.asarray(Wk), np.asarray(Wv),
                         np.asarray(Wo), np.asarray(Wphi), np.asarray(bphi))
    res = run_bass_kernel_spmd(nc, in_maps, list(range(8)), trace=_trace)
    _CACHE["last_res"] = res
    B = hidden_states.shape[0]
    out = np.empty((B, S, HID), dtype=np.float32)
    for b in range(B):
        acc = res.results[b * 4 + 0]["out"].astype(np.float32)
        for g in range(1, 4):
            acc = acc + res.results[b * 4 + g]["out"].astype(np.float32)
        out[b] = acc
    return out


# revision 13
# speedup vs baseline: 1.2080x; 1.0844x over previous
"""Trainium2 Bass kernel for LlamaRALAAttention (B=2, S=4096, HID=2048, NH=16, NKV=4, HD=128).

Sharding: 8 cores = DP(batch=2) x TP(kv-head groups=4). Core c handles batch c//4,
kv group c%4 (4 q heads + 1 kv head). Softmax/mean over S stay core-local.
o_proj partials are written bf16 and summed on host (the only cross-core reduction).

Pipeline (per core, "everything transposed" layout):
  xT [HID,S] host-pretransposed, bf16. Projections stream xT chunks as moving operand.
  q path in [d,s] layout: q^T = Wq_h^T @ xT, RoPE via R-matmul + cos/sin mults,
    kappa=exp(min(x,0))+max(x,0) -> QkT (bf16, resident); Qg partial folded into the
    kappa op via accum_out.
  k/v path in [s,d] layout: lhsT=xT tile (stationary), rhs=[Wk|Wv]; RoPE on free dim;
    kappa -> Kk_sd, v_sd (bf16, resident). KkT via PE transpose.
  All DVE-fed PE ops (KkT transposes, q-rope matmuls) go through a pending FIFO and are
  issued one matmul-group late, so the in-order PE queue never waits on a DVE chain.
  Logits: per-s-tile matvecs batched over the 4 heads into one PSUM bank; softmax
  (exact global max) batched over heads and interleaved into the first phi chunk;
  outer = (alpha*Kk)^T @ v with alpha applied via broadcast muls; result^T: lhsT=outer,
  rhs=QkT; ctx^T = phiT * result^T; o_proj one chunk behind phi/result so the ctx DVE
  muls never stall PE.
"""

import sys

sys.path.insert(0, "/opt/trn_rl_repo")

import numpy as np
import ml_dtypes

import concourse.bass as bass
import concourse.mybir as mybir
import concourse.tile as tile
from concourse import bacc
from concourse.bass_utils import run_bass_kernel_spmd
from concourse.masks import make_identity

P = 128
S = 4096
HID = 2048
HD = 128
NHL = 4            # q heads per core
KO = HID // P      # 16 contraction subtiles
CS = 512           # token chunk size
NCH = S // CS      # 8 chunks
NST = S // P       # 32 s-tiles
HSTEP = NST // 2
ROPE_THETA = 10000.0

F32 = mybir.dt.float32
BF16 = mybir.dt.bfloat16
BF = ml_dtypes.bfloat16

_CACHE = {}


def _build():
    nc = bacc.Bacc("TRN2", target_bir_lowering=False, debug=False, num_devices=8)

    xT = nc.dram_tensor("xT", [HID, S], BF16, kind="ExternalInput").ap()
    cosT = nc.dram_tensor("cosT", [P, S], F32, kind="ExternalInput").ap()
    sinT = nc.dram_tensor("sinT", [P, S], F32, kind="ExternalInput").ap()
    cos_sd = nc.dram_tensor("cos_sd", [S, HD], F32, kind="ExternalInput").ap()
    sin_sd = nc.dram_tensor("sin_sd", [S, HD], F32, kind="ExternalInput").ap()
    Wq = nc.dram_tensor("Wq", [HID, NHL * HD], BF16, kind="ExternalInput").ap()
    Wkv = nc.dram_tensor("Wkv", [HID, 2 * HD], BF16, kind="ExternalInput").ap()
    Wphi = nc.dram_tensor("Wphi", [HID, NHL * HD], BF16, kind="ExternalInput").ap()
    Wo = nc.dram_tensor("Wo", [NHL * HD, HID], BF16, kind="ExternalInput").ap()
    bphi = nc.dram_tensor("bphi", [NHL * HD], F32, kind="ExternalInput").ap()
    RT = nc.dram_tensor("RT", [P, P], BF16, kind="ExternalInput").ap()
    out = nc.dram_tensor("out", [S, HID], BF16, kind="ExternalOutput").ap()

    xT_r = xT.rearrange("(ko p) s -> p ko s", p=P)
    Wq_r = Wq.rearrange("(ko p) m -> p ko m", p=P)
    Wkv_r = Wkv.rearrange("(ko p) m -> p ko m", p=P)
    Wphi_r = Wphi.rearrange("(ko p) m -> p ko m", p=P)
    Wo_r = Wo.rearrange("(h p) n -> p h n", p=P)
    cos_sd_r = cos_sd.rearrange("(t p) d -> p t d", p=P)
    sin_sd_r = sin_sd.rearrange("(t p) d -> p t d", p=P)
    bphi_r = bphi.rearrange("(h p) -> p h", p=P)
    out_r = out.rearrange("(t p) n -> p t n", p=P)

    from contextlib import ExitStack
    with tile.TileContext(nc) as tc, ExitStack() as es:
        # ---- pools ----
        res = es.enter_context(tc.tile_pool(name="res", bufs=1))        # residents
        wts = es.enter_context(tc.tile_pool(name="wts", bufs=2))        # big weights, shared slots
        xp = es.enter_context(tc.tile_pool(name="xp", bufs=2))          # xT chunks
        stream = es.enter_context(tc.tile_pool(name="stream", bufs=2))  # big per-chunk tiles
        stream3 = es.enter_context(tc.tile_pool(name="stream3", bufs=3))  # small per-chunk tiles
        small = es.enter_context(tc.tile_pool(name="small", bufs=2))    # tiny tiles
        # PSUM: 8 banks total. pa: all projections (kv/q/phi); pr: rope+result;
        # po: o_proj out; pmix: transposes/logits/softmax/outer (strictly sequential).
        pa = es.enter_context(tc.tile_pool(name="pa", bufs=3, space="PSUM"))
        pr = es.enter_context(tc.tile_pool(name="pr", bufs=2, space="PSUM"))
        po = es.enter_context(tc.tile_pool(name="po", bufs=2, space="PSUM"))
        pmix = es.enter_context(tc.tile_pool(name="pmix", bufs=1, space="PSUM"))

        # ---- residents / weights ----
        Wkv_sb = res.tile([P, KO, 2 * HD], BF16)
        nc.sync.dma_start(Wkv_sb[:], Wkv_r)
        RT_sb = res.tile([P, P], BF16)
        nc.sync.dma_start(RT_sb[:], RT)
        bphi_sb = res.tile([P, NHL], F32)
        nc.sync.dma_start(bphi_sb[:], bphi_r)
        Wq_sb = wts.tile([P, KO, NHL * HD], BF16, tag="big")

        ident_bf = res.tile([P, P], BF16)
        make_identity(nc, ident_bf[:])
        ident_f32 = res.tile([P, P], F32)
        make_identity(nc, ident_f32[:])
        ones_f32 = res.tile([P, 1], F32)
        nc.vector.memset(ones_f32[:], 1.0)
        onesr_f32 = res.tile([1, P], F32)
        nc.vector.memset(onesr_f32[:], 1.0)
        negr_f32 = res.tile([1, P], F32)
        nc.vector.memset(negr_f32[:], -1.0)

        QkT = res.tile([P, NHL, S], BF16)       # 32KB/part
        KkT = res.tile([P, S], BF16)            # 8KB/part
        Kk_sd = res.tile([P, NST, HD], BF16)    # 8KB/part
        v_sd = res.tile([P, NST, HD], BF16)     # 8KB/part
        qg_parts = res.tile([P, NHL, NCH], F32)
        outer_bf = res.tile([P, NHL, HD], BF16)
        alpha_bf = res.tile([P, NHL, NST], BF16)
        logits_sd = res.tile([P, NHL, NST], F32)

        # ================= phase A =================
        # Pending PE work that depends on a DVE chain; each entry is issued one
        # matmul-group later so the in-order PE queue never stalls.
        pend = []

        def flush_pending():
            if pend:
                pend.pop(0)()

        def make_transpose(stg):
            def f():
                pst = pmix.tile([P, P], BF16, tag="mix", name="pst")
                nc.tensor.transpose(pst[:], Kk_sd[:, stg, :], ident_bf[:])
                nc.any.tensor_copy(KkT[:, stg * P:(stg + 1) * P], pst[:])
            return f

        def make_rope(h, c, psq, qs, cs_t):
            def f():
                psr = pr.tile([P, CS], F32, tag="psr", name="psr")
                nc.tensor.matmul(psr[:], RT_sb[:], qs[:], start=True, stop=True)
                qro = stream.tile([P, CS], F32, tag="qro", name="qro")
                nc.vector.tensor_mul(qro[:], psq[:], cs_t[:])
                nc.vector.tensor_add(qro[:], qro[:], psr[:])
                # kappa -> QkT, Qg partial folded in via accum_out
                mq = stream.tile([P, CS], F32, tag="mq", name="mq")
                nc.gpsimd.tensor_scalar_min(mq[:], qro[:], 0.0)
                eq = stream.tile([P, CS], F32, tag="eq", name="eq")
                nc.scalar.activation(eq[:], mq[:], mybir.ActivationFunctionType.Exp)
                nc.vector.scalar_tensor_tensor(
                    QkT[:, h, c * CS:(c + 1) * CS], qro[:], 0.0, eq[:],
                    mybir.AluOpType.max, mybir.AluOpType.add,
                    accum_out=qg_parts[:, h, c:c + 1])
            return f

        xt_c0_phaseC = [None]

        for c in range(NCH):
            xt = xp.tile([P, KO, CS], BF16, tag="xt", name="xt")
            if c == 0:
                # split so the first kv matmul group starts as early as possible
                for st in range(4):
                    nc.sync.dma_start(xt[:, :, st * P:(st + 1) * P],
                                      xT_r[:, :, st * P:(st + 1) * P])
            else:
                nc.sync.dma_start(xt[:, :, :CS // 2], xT_r[:, :, c * CS:c * CS + CS // 2])
                nc.sync.dma_start(xt[:, :, CS // 2:], xT_r[:, :, c * CS + CS // 2:(c + 1) * CS])
            csd = stream.tile([P, 4, HD], F32, tag="cossd", name="csd")
            nc.sync.dma_start(csd[:], cos_sd_r[:, c * 4:(c + 1) * 4, :])
            ssd = stream.tile([P, 4, HD], F32, tag="sinsd", name="ssd")
            nc.sync.dma_start(ssd[:], sin_sd_r[:, c * 4:(c + 1) * 4, :])
            if c == 0:
                nc.sync.dma_start(Wq_sb[:, :KO // 2, :], Wq_r[:, :KO // 2, :])
                nc.sync.dma_start(Wq_sb[:, KO // 2:, :], Wq_r[:, KO // 2:, :])
            cs_t = stream.tile([P, CS], F32, tag="cosT", name="cs_t")
            nc.sync.dma_start(cs_t[:], cosT[:, c * CS:(c + 1) * CS])
            sn_t = stream.tile([P, CS], F32, tag="sinT", name="sn_t")
            nc.sync.dma_start(sn_t[:], sinT[:, c * CS:(c + 1) * CS])
            if c == NCH - 1:
                # prefetch phase-C weights + first phase-C x chunk during the last
                # phase-A chunk's compute
                Wphi_sb = wts.tile([P, KO, NHL * HD], BF16, tag="big")
                nc.sync.dma_start(Wphi_sb[:], Wphi_r)
                xtc0 = xp.tile([P, KO, CS], BF16, tag="xt", name="xt")
                nc.sync.dma_start(xtc0[:, :, :CS // 2], xT_r[:, :, :CS // 2])
                nc.sync.dma_start(xtc0[:, :, CS // 2:], xT_r[:, :, CS // 2:CS])
                xt_c0_phaseC[0] = xtc0

            # ---- k + v for the 4 s-tiles of this chunk ----
            for st in range(4):
                stg = c * 4 + st
                pskv = pa.tile([P, 2 * HD], F32, tag="pa", name="pskv")
                for ko in range(KO):
                    nc.tensor.matmul(
                        pskv[:], xt[:, ko, st * P:(st + 1) * P], Wkv_sb[:, ko, :],
                        start=(ko == 0), stop=(ko == KO - 1))
                flush_pending()
                k_ps = pskv[:, :HD]
                nc.any.tensor_copy(v_sd[:, stg, :], pskv[:, HD:])
                # rope-k in [s,d]: rot on free halves
                kr = stream3.tile([P, HD], F32, tag="kr", name="kr")
                nc.vector.tensor_mul(kr[:], k_ps, csd[:, st, :])
                ta = stream3.tile([P, 64], F32, tag="ta", name="ta")
                nc.vector.tensor_mul(ta[:], k_ps[:, 64:], ssd[:, st, :64])
                nc.vector.tensor_sub(kr[:, :64], kr[:, :64], ta[:])
                tb = stream3.tile([P, 64], F32, tag="tb", name="tb")
                nc.vector.tensor_mul(tb[:], k_ps[:, :64], ssd[:, st, 64:])
                nc.vector.tensor_add(kr[:, 64:], kr[:, 64:], tb[:])
                # kappa
                mk = stream3.tile([P, HD], F32, tag="mk", name="mk")
                nc.gpsimd.tensor_scalar_min(mk[:], kr[:], 0.0)
                ek = stream3.tile([P, HD], F32, tag="ek", name="ek")
                nc.scalar.activation(ek[:], mk[:], mybir.ActivationFunctionType.Exp)
                nc.vector.scalar_tensor_tensor(
                    Kk_sd[:, stg, :], kr[:], 0.0, ek[:],
                    mybir.AluOpType.max, mybir.AluOpType.add)
                pend.append(make_transpose(stg))

            # ---- q heads ----
            for h in range(NHL):
                psq = pa.tile([P, CS], F32, tag="pa", name="psq")
                for ko in range(KO):
                    nc.tensor.matmul(
                        psq[:], Wq_sb[:, ko, h * HD:(h + 1) * HD], xt[:, ko, :],
                        start=(ko == 0), stop=(ko == KO - 1))
                flush_pending()
                # sin is 64-periodic over d, so rot(q)*sin == rot(q*sin):
                # multiply by sin BEFORE the rotation matmul (saves the psum copy)
                qs = stream3.tile([P, CS], BF16, tag="qbf", name="qs")
                nc.vector.tensor_mul(qs[:], psq[:], sn_t[:])
                pend.append(make_rope(h, c, psq, qs, cs_t))

        # ================= phase B + C =================
        Wo_sb = wts.tile([P, NHL, HID], BF16, tag="big")
        nc.sync.dma_start(Wo_sb[:], Wo_r)

        flush_pending()   # rope(h3, c7)

        # tiny sbuf tiles for the batched softmax
        qg_bf = small.tile([P, NHL], BF16, tag="qgbf")
        qg_f = small.tile([P, NHL], F32, tag="qgf")
        pmax4 = small.tile([P, NHL, 1], F32, tag="pmax4")
        gmax4 = small.tile([NHL, 1], F32, tag="gmax4")
        gms = small.tile([1, NHL], F32, tag="gms")
        ngm4 = small.tile([P, NHL], F32, tag="ngm4")
        e4 = small.tile([P, NHL, NST], F32, tag="e4", bufs=1)
        srow4 = small.tile([P, NHL], F32, tag="srow4")
        rcp4 = small.tile([NHL, 1], F32, tag="rcp4")
        rcs = small.tile([1, NHL], F32, tag="rcs")
        rcpb4 = small.tile([P, NHL], F32, tag="rcpb4")

        def issue_logits():
            # Qg finalize (DVE), then logits for all heads, accumulated into one bank
            for h in range(NHL):
                nc.vector.tensor_reduce(
                    qg_f[:, h:h + 1], qg_parts[:, h, :],
                    mybir.AxisListType.X, mybir.AluOpType.add)
            nc.vector.tensor_scalar_mul(qg_bf[:], qg_f[:], 1.0 / S)
            psl = pmix.tile([P, NST, NHL], F32, tag="mix", name="psl")
            for st in range(NST):
                nc.tensor.matmul(
                    psl[:, st, :], KkT[:, st * P:(st + 1) * P], qg_bf[:],
                    start=True, stop=True)
            nc.any.tensor_copy(logits_sd.rearrange("p h t -> p t h")[:], psl[:])
            nc.vector.tensor_reduce(
                pmax4[:], logits_sd[:], mybir.AxisListType.X, mybir.AluOpType.max)

        def issue_softmax1():
            # global max per head: transpose partial maxes, reduce, negate-broadcast
            pmt4 = pmix.tile([NHL, P], F32, tag="mix", name="pmt4")
            nc.tensor.transpose(pmt4[:], pmax4[:, :, 0], ident_f32[:])
            nc.vector.tensor_reduce(
                gmax4[:], pmt4[:], mybir.AxisListType.X, mybir.AluOpType.max)
            gmT = pmix.tile([1, NHL], F32, tag="mix", name="gmT")
            nc.tensor.transpose(gmT[:], gmax4[:], ident_f32[:NHL, :NHL])
            nc.vector.tensor_scalar_mul(gms[:], gmT[:], -1.0)
            pngm4 = pmix.tile([P, NHL], F32, tag="mix", name="pngm4")
            nc.tensor.matmul(pngm4[:], onesr_f32[:], gms[:], start=True, stop=True)
            nc.vector.tensor_copy(ngm4[:], pngm4[:])
            # e = exp(l - gmax) per head, row sums via accum_out
            for h in range(NHL):
                nc.scalar.activation(
                    e4[:, h, :], logits_sd[:, h, :], mybir.ActivationFunctionType.Exp,
                    bias=ngm4[:, h:h + 1], accum_out=srow4[:, h:h + 1])

        def issue_softmax2():
            # totals per head (fp32 matmul), reciprocal, broadcast, alpha = S * e / Z
            ptot4 = pmix.tile([NHL, 1], F32, tag="mix", name="ptot4")
            nc.tensor.matmul(ptot4[:], srow4[:], ones_f32[:], start=True, stop=True)
            nc.vector.reciprocal(rcp4[:], ptot4[:])
            rcT = pmix.tile([1, NHL], F32, tag="mix", name="rcT")
            nc.tensor.transpose(rcT[:], rcp4[:], ident_f32[:NHL, :NHL])
            nc.vector.tensor_copy(rcs[:], rcT[:])
            prc4 = pmix.tile([P, NHL], F32, tag="mix", name="prc4")
            nc.tensor.matmul(prc4[:], onesr_f32[:], rcs[:], start=True, stop=True)
            nc.vector.tensor_copy(rcpb4[:], prc4[:])
            nc.vector.scalar_tensor_tensor(
                alpha_bf[:], e4[:], float(S),
                rcpb4[:].unsqueeze(2).broadcast_to([P, NHL, NST]),
                mybir.AluOpType.mult, mybir.AluOpType.mult)

        def issue_outer(h):
            # alpha (broadcast over d) * Kk in two half-s blocks, accumulate outer
            pso = pmix.tile([P, HD], F32, tag="mix", name="pso")
            for half in range(2):
                st0 = half * HSTEP
                kka = stream3.tile([P, HSTEP, HD], BF16, tag="kka", name="kka", bufs=2)
                ab = alpha_bf[:, h, st0:st0 + HSTEP].unsqueeze(2).broadcast_to([P, HSTEP, HD])
                nc.vector.tensor_mul(kka[:], Kk_sd[:, st0:st0 + HSTEP, :], ab)
                for st in range(HSTEP):
                    nc.tensor.matmul(pso[:], kka[:, st, :], v_sd[:, st0 + st, :],
                                     start=(half == 0 and st == 0),
                                     stop=(half == 1 and st == HSTEP - 1))
            nc.any.tensor_copy(outer_bf[:, h, :], pso[:])

        copy_engines = [
            lambda dst, src: nc.vector.tensor_copy(dst, src),
            lambda dst, src: nc.scalar.copy(dst, src),
        ]
        _ctx_of = {}

        def oproj_group(c, st, n, pool, tag):
            stg = c * 4 + st
            pso2 = pool.tile([P, 512], F32, tag=tag, name="pso2")
            for h in range(NHL):
                nc.tensor.matmul(
                    pso2[:], _ctx_of[c][:, h, st * P:(st + 1) * P],
                    Wo_sb[:, h, n * 512:(n + 1) * 512],
                    start=(h == 0), stop=(h == NHL - 1))
            ob = stream.tile([P, 512], BF16, tag="ob", name="ob", bufs=4)
            copy_engines[(st + n) % 2](ob[:], pso2[:])
            nc.sync.dma_start(out_r[:, stg, n * 512:(n + 1) * 512], ob[:])

        def issue_oproj(c, groups=range(16)):
            for g in groups:
                oproj_group(c, g // 4, g % 4, po, "psout")

        for c in range(NCH):
            if c == 0:
                xt = xt_c0_phaseC[0]
            else:
                xt = xp.tile([P, KO, CS], BF16, tag="xt", name="xt")
                nc.sync.dma_start(xt[:, :, :CS // 2], xT_r[:, :, c * CS:c * CS + CS // 2])
                nc.sync.dma_start(xt[:, :, CS // 2:], xT_r[:, :, c * CS + CS // 2:(c + 1) * CS])
            ctx_bf = stream.tile([P, NHL, CS], BF16, tag="ctx", name="ctx_bf")
            _ctx_of[c] = ctx_bf
            for h in range(NHL):
                psp = pa.tile([P, CS], F32, tag="pa", name="psp")
                for ko in range(KO):
                    nc.tensor.matmul(
                        psp[:], Wphi_sb[:, ko, h * HD:(h + 1) * HD], xt[:, ko, :],
                        start=(ko == 0), stop=(ko == KO - 1))
                if c == 0:
                    # softmax + outer ride inside chunk 0's phi matmul groups
                    if h == 0:
                        issue_logits()
                    elif h == 1:
                        issue_softmax1()
                    elif h == 2:
                        issue_softmax2()
                else:
                    # previous chunk's o_proj rides between this chunk's phi
                    # head-groups so its psum->sbuf copies never stall PE
                    issue_oproj(c - 1, range(h * 4, h * 4 + 4))
                phiT = stream.tile([P, CS], F32, tag="phiT", name="phiT", bufs=4)
                nc.scalar.activation(phiT[:], psp[:], mybir.ActivationFunctionType.Identity,
                                     bias=bphi_sb[:, h:h + 1])
                if c > 0:
                    psr = pr.tile([P, CS], F32, tag="psr", name="psr")
                    nc.tensor.matmul(psr[:], outer_bf[:, h, :],
                                     QkT[:, h, c * CS:(c + 1) * CS], start=True, stop=True)
                    nc.vector.tensor_mul(ctx_bf[:, h, :], phiT[:], psr[:])
                else:
                    _phiT_c0 = _ctx_of.setdefault("phiT_c0", [])
                    _phiT_c0.append(phiT)
            if c == 0:
                for h in range(NHL):
                    issue_outer(h)
                for h in range(NHL):
                    psr = pr.tile([P, CS], F32, tag="psr", name="psr")
                    nc.tensor.matmul(psr[:], outer_bf[:, h, :],
                                     QkT[:, h, :CS], start=True, stop=True)
                    nc.vector.tensor_mul(ctx_bf[:, h, :], _ctx_of["phiT_c0"][h][:], psr[:])
        # final chunk's o_proj: alternate psum between po and the now-idle pa pool
        # for deeper buffering (the copy latency never blocks the matmuls)
        for g in range(16):
            oproj_group(NCH - 1, g // 4, g % 4, (po, pa)[g % 2], ("psout", "pa")[g % 2])

    nc.compile()
    return nc


def _host_prep(hidden_states, position_ids, Wq, Wk, Wv, Wo, Wphi, bphi):
    B = hidden_states.shape[0]
    # rope tables (match reference fp32 math)
    inv_freq = (1.0 / (ROPE_THETA ** (np.arange(0, HD, 2, dtype=np.float32) / HD))).astype(np.float32)
    in_maps = []
    Rm = np.zeros((P, P), dtype=np.float32)
    Rm[np.arange(64), np.arange(64) + 64] = -1.0
    Rm[np.arange(64) + 64, np.arange(64)] = 1.0
    RT_np = np.ascontiguousarray(Rm.T).astype(BF)
    for b in range(B):
        freqs = position_ids[b].astype(np.float32)[:, None] * inv_freq[None, :]
        emb = np.concatenate([freqs, freqs], axis=1)          # [S, 128]
        cos_b = np.cos(emb).astype(np.float32)
        sin_b = np.sin(emb).astype(np.float32)
        xT_b = np.ascontiguousarray(hidden_states[b].T).astype(BF)
        cosT_b = np.ascontiguousarray(cos_b.T)
        sinT_b = np.ascontiguousarray(sin_b.T)
        for g in range(4):
            sl4 = slice(g * 512, (g + 1) * 512)
            sl1 = slice(g * 128, (g + 1) * 128)
            in_maps.append({
                "xT": xT_b,
                "cosT": cosT_b, "sinT": sinT_b,
                "cos_sd": cos_b, "sin_sd": sin_b,
                "Wq": np.ascontiguousarray(Wq[:, sl4]).astype(BF),
                "Wkv": np.ascontiguousarray(
                    np.concatenate([Wk[:, sl1], Wv[:, sl1]], axis=1)).astype(BF),
                "Wphi": np.ascontiguousarray(Wphi[:, sl4]).astype(BF),
                "Wo": np.ascontiguousarray(Wo[sl4, :]).astype(BF),
                "bphi": np.ascontiguousarray(bphi[sl4]).astype(np.float32),
                "RT": RT_np,
            })
    return in_maps


def kernel(hidden_states, position_ids, Wq, Wk, Wv, Wo, Wphi, bphi, _trace=False):
    if "nc" not in _CACHE:
        _CACHE["nc"] = _build()
    nc = _CACHE["nc"]
    in_maps = _host_prep(np.asarray(hidden_states), np.asarray(position_ids),
                         np.asarray(Wq), np.asarray(Wk), np.asarray(Wv),
                         np.asarray(Wo), np.asarray(Wphi), np.asarray(bphi))
    res = run_bass_kernel_spmd(nc, in_maps, list(range(8)), trace=_trace)
    _CACHE["last_res"] = res
    B = hidden_states.shape[0]
    out = np.empty((B, S, HID), dtype=np.float32)
    for b in range(B):
        acc = res.results[b * 4 + 0]["out"].astype(np.float32)
        for g in range(1, 4):
            acc = acc + res.results[b * 4 + g]["out"].astype(np.float32)
        out[b] = acc
    return out


# revision 22
# speedup vs baseline: 1.2140x; 1.0050x over previous
"""Trainium2 Bass kernel for LlamaRALAAttention (B=2, S=4096, HID=2048, NH=16, NKV=4, HD=128).

Sharding: 8 cores = DP(batch=2) x TP(kv-head groups=4). Core c handles batch c//4,
kv group c%4 (4 q heads + 1 kv head). Softmax/mean over S stay core-local.
o_proj partials are written bf16 and summed on host (the only cross-core reduction).

Pipeline (per core, "everything transposed" layout):
  xT [HID,S] host-pretransposed, bf16. Projections stream xT chunks as moving operand.
  q path in [d,s] layout: q^T = Wq_h^T @ xT, RoPE via R-matmul + cos/sin mults,
    kappa=exp(min(x,0))+max(x,0) -> QkT (bf16, resident); Qg partial folded into the
    kappa op via accum_out.
  k/v path in [s,d] layout: lhsT=xT tile (stationary), rhs=[Wk|Wv]; RoPE on free dim;
    kappa -> Kk_sd, v_sd (bf16, resident). KkT via PE transpose.
  All DVE-fed PE ops (KkT transposes, q-rope matmuls) go through a pending FIFO and are
  issued one matmul-group late, so the in-order PE queue never waits on a DVE chain.
  Logits: per-s-tile matvecs batched over the 4 heads into one PSUM bank; softmax
  (exact global max) batched over heads and interleaved into the first phi chunk;
  outer = (alpha*Kk)^T @ v with alpha applied via broadcast muls; result^T: lhsT=outer,
  rhs=QkT; ctx^T = phiT * result^T; o_proj one chunk behind phi/result so the ctx DVE
  muls never stall PE.
"""

import sys

sys.path.insert(0, "/opt/trn_rl_repo")

import numpy as np
import ml_dtypes

import concourse.bass as bass
import concourse.mybir as mybir
import concourse.tile as tile
from concourse import bacc
from concourse.bass_utils import run_bass_kernel_spmd
from concourse.masks import make_identity

P = 128
S = 4096
HID = 2048
HD = 128
NHL = 4            # q heads per core
KO = HID // P      # 16 contraction subtiles
CS = 512           # token chunk size
NCH = S // CS      # 8 chunks
NST = S // P       # 32 s-tiles
HSTEP = NST // 2
ROPE_THETA = 10000.0

F32 = mybir.dt.float32
BF16 = mybir.dt.bfloat16
BF = ml_dtypes.bfloat16

_CACHE = {}


def _build():
    nc = bacc.Bacc("TRN2", target_bir_lowering=False, debug=False, num_devices=8)

    xT = nc.dram_tensor("xT", [HID, S], BF16, kind="ExternalInput").ap()
    cosT = nc.dram_tensor("cosT", [P, S], F32, kind="ExternalInput").ap()
    sinT = nc.dram_tensor("sinT", [P, S], F32, kind="ExternalInput").ap()
    cos_sd = nc.dram_tensor("cos_sd", [S, HD], F32, kind="ExternalInput").ap()
    sin_sd = nc.dram_tensor("sin_sd", [S, HD], F32, kind="ExternalInput").ap()
    Wq = nc.dram_tensor("Wq", [HID, NHL * HD], BF16, kind="ExternalInput").ap()
    Wkv = nc.dram_tensor("Wkv", [HID, 2 * HD], BF16, kind="ExternalInput").ap()
    Wphi = nc.dram_tensor("Wphi", [HID, NHL * HD], BF16, kind="ExternalInput").ap()
    Wo = nc.dram_tensor("Wo", [NHL * HD, HID], BF16, kind="ExternalInput").ap()
    bphi = nc.dram_tensor("bphi", [NHL * HD], F32, kind="ExternalInput").ap()
    RT = nc.dram_tensor("RT", [P, P], BF16, kind="ExternalInput").ap()
    out = nc.dram_tensor("out", [S, HID], BF16, kind="ExternalOutput").ap()

    xT_r = xT.rearrange("(ko p) s -> p ko s", p=P)
    Wq_r = Wq.rearrange("(ko p) m -> p ko m", p=P)
    Wkv_r = Wkv.rearrange("(ko p) m -> p ko m", p=P)
    Wphi_r = Wphi.rearrange("(ko p) m -> p ko m", p=P)
    Wo_r = Wo.rearrange("(h p) n -> p h n", p=P)
    cos_sd_r = cos_sd.rearrange("(t p) d -> p t d", p=P)
    sin_sd_r = sin_sd.rearrange("(t p) d -> p t d", p=P)
    bphi_r = bphi.rearrange("(h p) -> p h", p=P)
    out_r = out.rearrange("(t p) n -> p t n", p=P)

    from contextlib import ExitStack
    with tile.TileContext(nc) as tc, ExitStack() as es:
        # ---- pools ----
        res = es.enter_context(tc.tile_pool(name="res", bufs=1))        # residents
        wts = es.enter_context(tc.tile_pool(name="wts", bufs=2))        # big weights, shared slots
        xp = es.enter_context(tc.tile_pool(name="xp", bufs=3))          # xT chunks
        stream = es.enter_context(tc.tile_pool(name="stream", bufs=2))  # big per-chunk tiles
        stream3 = es.enter_context(tc.tile_pool(name="stream3", bufs=3))  # small per-chunk tiles
        small = es.enter_context(tc.tile_pool(name="small", bufs=2))    # tiny tiles
        # PSUM: 8 banks total. pa: all projections (kv/q/phi); pr: rope+result;
        # po: o_proj out; pmix: transposes/logits/softmax/outer (strictly sequential).
        pa = es.enter_context(tc.tile_pool(name="pa", bufs=3, space="PSUM"))
        pr = es.enter_context(tc.tile_pool(name="pr", bufs=2, space="PSUM"))
        po = es.enter_context(tc.tile_pool(name="po", bufs=2, space="PSUM"))
        pmix = es.enter_context(tc.tile_pool(name="pmix", bufs=1, space="PSUM"))

        # ---- residents / weights ----
        Wkv_sb = res.tile([P, KO, 2 * HD], BF16)
        nc.sync.dma_start(Wkv_sb[:, :KO // 2, :], Wkv_r[:, :KO // 2, :])
        nc.sync.dma_start(Wkv_sb[:, KO // 2:, :], Wkv_r[:, KO // 2:, :])
        RT_sb = res.tile([P, P], BF16)
        bphi_sb = res.tile([P, NHL], F32)
        Wq_sb = wts.tile([P, KO, NHL * HD], BF16, tag="big")

        ident_bf = res.tile([P, P], BF16)
        make_identity(nc, ident_bf[:])
        ident_f32 = res.tile([P, P], F32)
        make_identity(nc, ident_f32[:])
        ones_f32 = res.tile([P, 1], F32)
        nc.vector.memset(ones_f32[:], 1.0)
        onesr_f32 = res.tile([1, P], F32)
        nc.vector.memset(onesr_f32[:], 1.0)
        negr_f32 = res.tile([1, P], F32)
        nc.vector.memset(negr_f32[:], -1.0)

        QkT = res.tile([P, NHL, S], BF16)       # 32KB/part
        KkT = res.tile([P, S], BF16)            # 8KB/part
        Kk_sd = res.tile([P, NST, HD], BF16)    # 8KB/part
        v_sd = res.tile([P, NST, HD], BF16)     # 8KB/part
        qg_parts = res.tile([P, NHL, NCH], F32)
        outer_bf = res.tile([P, NHL, HD], BF16)
        alpha_bf = res.tile([P, NHL, NST], BF16)
        logits_sd = res.tile([P, NHL, NST], F32)

        # ================= phase A =================
        # Pending PE work that depends on a DVE chain; each entry is issued one
        # matmul-group later so the in-order PE queue never stalls.
        pend = []

        def flush_pending():
            if pend:
                pend.pop(0)()

        def make_transpose(stg):
            def f():
                pst = pmix.tile([P, P], BF16, tag="mix", name="pst")
                nc.tensor.transpose(pst[:], Kk_sd[:, stg, :], ident_bf[:])
                nc.any.tensor_copy(KkT[:, stg * P:(stg + 1) * P], pst[:])
            return f

        def make_rope(h, c, psq, qs, cs_t):
            def f():
                psr = pr.tile([P, CS], F32, tag="psr", name="psr")
                nc.tensor.matmul(psr[:], RT_sb[:], qs[:], start=True, stop=True)
                qro = stream.tile([P, CS], F32, tag="qro", name="qro")
                nc.vector.tensor_mul(qro[:], psq[:], cs_t[:])
                nc.vector.tensor_add(qro[:], qro[:], psr[:])
                # kappa -> QkT, Qg partial folded in via accum_out
                mq = stream.tile([P, CS], BF16, tag="mq", name="mq")
                nc.gpsimd.tensor_scalar_min(mq[:], qro[:], 0.0)
                eq = stream.tile([P, CS], BF16, tag="eq", name="eq")
                nc.scalar.activation(eq[:], mq[:], mybir.ActivationFunctionType.Exp)
                nc.vector.scalar_tensor_tensor(
                    QkT[:, h, c * CS:(c + 1) * CS], qro[:], 0.0, eq[:],
                    mybir.AluOpType.max, mybir.AluOpType.add,
                    accum_out=qg_parts[:, h, c:c + 1])
            return f

        xt_c0_phaseC = [None]

        for c in range(NCH):
            xt = xp.tile([P, KO, CS], BF16, tag="xt", name="xt")
            nc.sync.dma_start(xt[:, :, :CS // 2], xT_r[:, :, c * CS:c * CS + CS // 2])
            nc.sync.dma_start(xt[:, :, CS // 2:], xT_r[:, :, c * CS + CS // 2:(c + 1) * CS])
            # cos/sin for k-rope: halves along d are equal, load 64 wide
            csd = stream.tile([P, 4, 64], F32, tag="cossd", name="csd")
            nc.sync.dma_start(csd[:], cos_sd_r[:, c * 4:(c + 1) * 4, :64])
            ssd = stream.tile([P, 4, 64], F32, tag="sinsd", name="ssd")
            nc.sync.dma_start(ssd[:], sin_sd_r[:, c * 4:(c + 1) * 4, :64])
            if c == 0:
                nc.sync.dma_start(Wq_sb[:, :KO // 2, :], Wq_r[:, :KO // 2, :])
                nc.sync.dma_start(Wq_sb[:, KO // 2:, :], Wq_r[:, KO // 2:, :])
            cs_t = stream.tile([P, CS], F32, tag="cosT", name="cs_t")
            nc.sync.dma_start(cs_t[:], cosT[:, c * CS:(c + 1) * CS])
            sn_t = stream.tile([P, CS], F32, tag="sinT", name="sn_t")
            nc.sync.dma_start(sn_t[:], sinT[:, c * CS:(c + 1) * CS])
            if c == 0:
                # deferred residents: not needed until the first rope / phase C
                nc.sync.dma_start(RT_sb[:], RT)
                nc.sync.dma_start(bphi_sb[:], bphi_r)
            if c == NCH - 1:
                # prefetch phase-C weights + first phase-C x chunk during the last
                # phase-A chunk's compute
                Wphi_sb = wts.tile([P, KO, NHL * HD], BF16, tag="big")
                nc.sync.dma_start(Wphi_sb[:], Wphi_r)
                xtc0 = xp.tile([P, KO, CS], BF16, tag="xt", name="xt")
                nc.sync.dma_start(xtc0[:, :, :CS // 2], xT_r[:, :, :CS // 2])
                nc.sync.dma_start(xtc0[:, :, CS // 2:], xT_r[:, :, CS // 2:CS])
                xt_c0_phaseC[0] = xtc0

            # ---- k + v for the 4 s-tiles of this chunk ----
            for st in range(4):
                stg = c * 4 + st
                pskv = pa.tile([P, 2 * HD], F32, tag="pa", name="pskv")
                for ko in range(KO):
                    nc.tensor.matmul(
                        pskv[:], xt[:, ko, st * P:(st + 1) * P], Wkv_sb[:, ko, :],
                        start=(ko == 0), stop=(ko == KO - 1))
                flush_pending()
                k_ps = pskv[:, :HD]
                nc.any.tensor_copy(v_sd[:, stg, :], pskv[:, HD:])
                # rope-k in [s,d]: rot on free halves (cos/sin halves along d are
                # equal, so broadcast the 64-wide tables)
                csb = csd[:, st, :].unsqueeze(1).broadcast_to([P, 2, 64])
                kr = stream3.tile([P, HD], F32, tag="kr", name="kr")
                nc.vector.tensor_mul(kr[:].rearrange("p (two d) -> p two d", two=2),
                                     k_ps.rearrange("p (two d) -> p two d", two=2), csb)
                ta = stream3.tile([P, 64], F32, tag="ta", name="ta")
                nc.vector.tensor_mul(ta[:], k_ps[:, 64:], ssd[:, st, :])
                nc.vector.tensor_sub(kr[:, :64], kr[:, :64], ta[:])
                tb = stream3.tile([P, 64], F32, tag="tb", name="tb")
                nc.vector.tensor_mul(tb[:], k_ps[:, :64], ssd[:, st, :])
                nc.vector.tensor_add(kr[:, 64:], kr[:, 64:], tb[:])
                # kappa
                mk = stream3.tile([P, HD], F32, tag="mk", name="mk")
                nc.gpsimd.tensor_scalar_min(mk[:], kr[:], 0.0)
                ek = stream3.tile([P, HD], F32, tag="ek", name="ek")
                nc.scalar.activation(ek[:], mk[:], mybir.ActivationFunctionType.Exp)
                nc.vector.scalar_tensor_tensor(
                    Kk_sd[:, stg, :], kr[:], 0.0, ek[:],
                    mybir.AluOpType.max, mybir.AluOpType.add)
                pend.append(make_transpose(stg))

            # ---- q heads ----
            for h in range(NHL):
                psq = pa.tile([P, CS], F32, tag="pa", name="psq")
                for ko in range(KO):
                    nc.tensor.matmul(
                        psq[:], Wq_sb[:, ko, h * HD:(h + 1) * HD], xt[:, ko, :],
                        start=(ko == 0), stop=(ko == KO - 1))
                flush_pending()
                # sin is 64-periodic over d, so rot(q)*sin == rot(q*sin):
                # multiply by sin BEFORE the rotation matmul (saves the psum copy)
                qs = stream3.tile([P, CS], BF16, tag="qbf", name="qs")
                nc.vector.tensor_mul(qs[:], psq[:], sn_t[:])
                pend.append(make_rope(h, c, psq, qs, cs_t))

        # ================= phase B + C =================
        Wo_sb = wts.tile([P, NHL, HID], BF16, tag="big")
        nc.sync.dma_start(Wo_sb[:], Wo_r)

        flush_pending()   # rope(h3, c7)

        # tiny sbuf tiles for the batched softmax
        qg_bf = small.tile([P, NHL], BF16, tag="qgbf")
        qg_f = small.tile([P, NHL], F32, tag="qgf")
        pmax4 = small.tile([P, NHL, 1], F32, tag="pmax4")
        gmax4 = small.tile([NHL, 1], F32, tag="gmax4")
        gms = small.tile([1, NHL], F32, tag="gms")
        ngm4 = small.tile([P, NHL], F32, tag="ngm4")
        e4 = small.tile([P, NHL, NST], F32, tag="e4", bufs=1)
        srow4 = small.tile([P, NHL], F32, tag="srow4")
        rcp4 = small.tile([NHL, 1], F32, tag="rcp4")
        rcs = small.tile([1, NHL], F32, tag="rcs")
        rcpb4 = small.tile([P, NHL], F32, tag="rcpb4")

        def issue_logits():
            # Qg finalize (DVE), then logits for all heads, accumulated into one bank
            for h in range(NHL):
                nc.vector.tensor_reduce(
                    qg_f[:, h:h + 1], qg_parts[:, h, :],
                    mybir.AxisListType.X, mybir.AluOpType.add)
            nc.vector.tensor_scalar_mul(qg_bf[:], qg_f[:], 1.0 / S)
            psl = pmix.tile([P, NST, NHL], F32, tag="mix", name="psl")
            for st in range(NST):
                nc.tensor.matmul(
                    psl[:, st, :], KkT[:, st * P:(st + 1) * P], qg_bf[:],
                    start=True, stop=True)
            nc.any.tensor_copy(logits_sd.rearrange("p h t -> p t h")[:], psl[:])
            nc.vector.tensor_reduce(
                pmax4[:], logits_sd[:], mybir.AxisListType.X, mybir.AluOpType.max)

        def issue_softmax1():
            # global max per head: transpose partial maxes, reduce, negate-broadcast
            pmt4 = pmix.tile([NHL, P], F32, tag="mix", name="pmt4")
            nc.tensor.transpose(pmt4[:], pmax4[:, :, 0], ident_f32[:])
            nc.vector.tensor_reduce(
                gmax4[:], pmt4[:], mybir.AxisListType.X, mybir.AluOpType.max)
            gmT = pmix.tile([1, NHL], F32, tag="mix", name="gmT")
            nc.tensor.transpose(gmT[:], gmax4[:], ident_f32[:NHL, :NHL])
            nc.vector.tensor_scalar_mul(gms[:], gmT[:], -1.0)
            pngm4 = pmix.tile([P, NHL], F32, tag="mix", name="pngm4")
            nc.tensor.matmul(pngm4[:], onesr_f32[:], gms[:], start=True, stop=True)
            nc.vector.tensor_copy(ngm4[:], pngm4[:])
            # e = exp(l - gmax) per head, row sums via accum_out
            for h in range(NHL):
                nc.scalar.activation(
                    e4[:, h, :], logits_sd[:, h, :], mybir.ActivationFunctionType.Exp,
                    bias=ngm4[:, h:h + 1], accum_out=srow4[:, h:h + 1])

        def issue_softmax2():
            # totals per head (fp32 matmul), reciprocal, broadcast, alpha = S * e / Z
            ptot4 = pmix.tile([NHL, 1], F32, tag="mix", name="ptot4")
            nc.tensor.matmul(ptot4[:], srow4[:], ones_f32[:], start=True, stop=True)
            nc.vector.reciprocal(rcp4[:], ptot4[:])
            rcT = pmix.tile([1, NHL], F32, tag="mix", name="rcT")
            nc.tensor.transpose(rcT[:], rcp4[:], ident_f32[:NHL, :NHL])
            nc.vector.tensor_copy(rcs[:], rcT[:])
            prc4 = pmix.tile([P, NHL], F32, tag="mix", name="prc4")
            nc.tensor.matmul(prc4[:], onesr_f32[:], rcs[:], start=True, stop=True)
            nc.vector.tensor_copy(rcpb4[:], prc4[:])
            nc.vector.scalar_tensor_tensor(
                alpha_bf[:], e4[:], float(S),
                rcpb4[:].unsqueeze(2).broadcast_to([P, NHL, NST]),
                mybir.AluOpType.mult, mybir.AluOpType.mult)

        def issue_outer(h):
            # alpha (broadcast over d) * Kk in quarter-s blocks, accumulate outer
            QSTEP = NST // 4
            pso = pmix.tile([P, HD], F32, tag="mix", name="pso")
            for quarter in range(4):
                st0 = quarter * QSTEP
                kka = stream3.tile([P, QSTEP, HD], BF16, tag="kka", name="kka", bufs=2)
                ab = alpha_bf[:, h, st0:st0 + QSTEP].unsqueeze(2).broadcast_to([P, QSTEP, HD])
                nc.vector.tensor_mul(kka[:], Kk_sd[:, st0:st0 + QSTEP, :], ab)
                for st in range(QSTEP):
                    nc.tensor.matmul(pso[:], kka[:, st, :], v_sd[:, st0 + st, :],
                                     start=(quarter == 0 and st == 0),
                                     stop=(quarter == 3 and st == QSTEP - 1))
            nc.any.tensor_copy(outer_bf[:, h, :], pso[:])

        copy_engines = [
            lambda dst, src: nc.vector.tensor_copy(dst, src),
            lambda dst, src: nc.scalar.copy(dst, src),
        ]
        _ctx_of = {}

        _ob4 = [None]

        def oproj_group(c, st, n, pool, tag):
            stg = c * 4 + st
            pso2 = pool.tile([P, 512], F32, tag=tag, name="pso2")
            for h in range(NHL):
                nc.tensor.matmul(
                    pso2[:], _ctx_of[c][:, h, st * P:(st + 1) * P],
                    Wo_sb[:, h, n * 512:(n + 1) * 512],
                    start=(h == 0), stop=(h == NHL - 1))
            if n == 0:
                _ob4[0] = stream.tile([P, HID], BF16, tag="ob", name="ob", bufs=2)
            copy_engines[(st + n) % 2](_ob4[0][:, n * 512:(n + 1) * 512], pso2[:])
            if n == 3:
                nc.sync.dma_start(out_r[:, stg, :], _ob4[0][:])

        def issue_oproj(c, groups=range(16)):
            for g in groups:
                oproj_group(c, g // 4, g % 4, po, "psout")

        for c in range(NCH):
            if c == 0:
                xt = xt_c0_phaseC[0]
            else:
                xt = xp.tile([P, KO, CS], BF16, tag="xt", name="xt")
                nc.sync.dma_start(xt[:, :, :CS // 2], xT_r[:, :, c * CS:c * CS + CS // 2])
                nc.sync.dma_start(xt[:, :, CS // 2:], xT_r[:, :, c * CS + CS // 2:(c + 1) * CS])
            ctx_bf = stream.tile([P, NHL, CS], BF16, tag="ctx", name="ctx_bf")
            _ctx_of[c] = ctx_bf
            for h in range(NHL):
                psp = pa.tile([P, CS], F32, tag="pa", name="psp")
                for ko in range(KO):
                    nc.tensor.matmul(
                        psp[:], Wphi_sb[:, ko, h * HD:(h + 1) * HD], xt[:, ko, :],
                        start=(ko == 0), stop=(ko == KO - 1))
                if c == 0:
                    # softmax + outer ride inside chunk 0's phi matmul groups
                    if h == 0:
                        issue_logits()
                    elif h == 1:
                        issue_softmax1()
                    elif h == 2:
                        issue_softmax2()
                else:
                    # previous chunk's o_proj rides between this chunk's phi
                    # head-groups so its psum->sbuf copies never stall PE
                    issue_oproj(c - 1, range(h * 4, h * 4 + 4))
                phiT = stream.tile([P, CS], BF16, tag="phiT", name="phiT", bufs=4)
                nc.scalar.activation(phiT[:], psp[:], mybir.ActivationFunctionType.Identity,
                                     bias=bphi_sb[:, h:h + 1])
                if c > 0:
                    psr = pr.tile([P, CS], F32, tag="psr", name="psr")
                    nc.tensor.matmul(psr[:], outer_bf[:, h, :],
                                     QkT[:, h, c * CS:(c + 1) * CS], start=True, stop=True)
                    nc.vector.tensor_mul(ctx_bf[:, h, :], phiT[:], psr[:])
                else:
                    _phiT_c0 = _ctx_of.setdefault("phiT_c0", [])
                    _phiT_c0.append(phiT)
            if c == 0:
                for h in range(NHL):
                    issue_outer(h)
                for h in range(NHL):
                    psr = pr.tile([P, CS], F32, tag="psr", name="psr")
                    nc.tensor.matmul(psr[:], outer_bf[:, h, :],
                                     QkT[:, h, :CS], start=True, stop=True)
                    nc.vector.tensor_mul(ctx_bf[:, h, :], _ctx_of["phiT_c0"][h][:], psr[:])
        # final chunk's o_proj: alternate psum between po and the now-idle pa pool
        # for deeper buffering (the copy latency never blocks the matmuls)
        for g in range(16):
            oproj_group(NCH - 1, g // 4, g % 4, (po, pa)[g % 2], ("psout", "pa")[g % 2])

    nc.compile()
    return nc


def _host_prep(hidden_states, position_ids, Wq, Wk, Wv, Wo, Wphi, bphi):
    B = hidden_states.shape[0]
    # rope tables (match reference fp32 math)
    inv_freq = (1.0 / (ROPE_THETA ** (np.arange(0, HD, 2, dtype=np.float32) / HD))).astype(np.float32)
    in_maps = []
    Rm = np.zeros((P, P), dtype=np.float32)
    Rm[np.arange(64), np.arange(64) + 64] = -1.0
    Rm[np.arange(64) + 64, np.arange(64)] = 1.0
    RT_np = np.ascontiguousarray(Rm.T).astype(BF)
    for b in range(B):
        freqs = position_ids[b].astype(np.float32)[:, None] * inv_freq[None, :]
        emb = np.concatenate([freqs, freqs], axis=1)          # [S, 128]
        cos_b = np.cos(emb).astype(np.float32)
        sin_b = np.sin(emb).astype(np.float32)
        xT_b = np.ascontiguousarray(hidden_states[b].T).astype(BF)
        cosT_b = np.ascontiguousarray(cos_b.T)
        sinT_b = np.ascontiguousarray(sin_b.T)
        for g in range(4):
            sl4 = slice(g * 512, (g + 1) * 512)
            sl1 = slice(g * 128, (g + 1) * 128)
            in_maps.append({
                "xT": xT_b,
                "cosT": cosT_b, "sinT": sinT_b,
                "cos_sd": cos_b, "sin_sd": sin_b,
                "Wq": np.ascontiguousarray(Wq[:, sl4]).astype(BF),
                "Wkv": np.ascontiguousarray(
                    np.concatenate([Wk[:, sl1], Wv[:, sl1]], axis=1)).astype(BF),
                "Wphi": np.ascontiguousarray(Wphi[:, sl4]).astype(BF),
                "Wo": np.ascontiguousarray(Wo[sl4, :]).astype(BF),
                "bphi": np.ascontiguousarray(bphi[sl4]).astype(np.float32),
                "RT": RT_np,
            })
    return in_maps


def kernel(hidden_states, position_ids, Wq, Wk, Wv, Wo, Wphi, bphi, _trace=False):
    if "nc" not in _CACHE:
        _CACHE["nc"] = _build()
    nc = _CACHE["nc"]
    in_maps = _host_prep(np.asarray(hidden_states), np.asarray(position_ids),
                         np.asarray(Wq), np.asarray(Wk), np.asarray(Wv),
                         np.asarray(Wo), np.asarray(Wphi), np.asarray(bphi))
    res = run_bass_kernel_spmd(nc, in_maps, list(range(8)), trace=_trace)
    _CACHE["last_res"] = res
    B = hidden_states.shape[0]
    out = np.empty((B, S, HID), dtype=np.float32)
    for b in range(B):
        acc = res.results[b * 4 + 0]["out"].astype(np.float32)
        for g in range(1, 4):
            acc = acc + res.results[b * 4 + g]["out"].astype(np.float32)
        out[b] = acc
    return out
